# revision 3
# baseline (speedup 1.0000x reference)
"""DensityExtractor (NeRF volume-rendering weights) Bass kernel for 8 TRN2 cores.

reference:
  dists[s] = d[s+1]-d[s] (last 1e10), scaled by |ray_dir|
  alpha = 1 - exp(-relu(rf[...,3]) * dists)
  weights = alpha * cumprod_exclusive(1 - alpha + 1e-10)

Reformulation (eps dropped; diff <= S*1e-10):
  m[s] = -relu(sigma[s]) * dists[s] * |dir|   (<= 0, clamped >= -6000*|dir|)
  T[s] = exp(inclusive_cumsum(m)[s])
  w[s] = T[s-1] - T[s],  w[0] = 1 - T[0]

v3: the cumsum runs on the tensor engine. Per 512-ray block (ray =
64p + j, 4 j-groups of 128 cols): PE-transpose m16 (fp16) per group ->
PSUM, copy to SBUF, then ONE matmul per group with the transposed m as
the STATIONARY and the upper-triangular ones as MOVING - the output
lands directly in ray-major [r, (g, s)] layout, so there is no
transpose-back. All stages are emitted software-pipelined (stage lag
per block) so no engine's in-order stream embeds a cross-engine wait.

Engine budget per block (us): gpsimd 1.5 (diff+memset+fix), DVE 2.2
(stt relu*dist, stt scale+clamp, final diff), scalar 2.0 (dv DMA issue,
PSUM copy, exp), PE ~1.1, sync 1.3 (rf issue + store). DMA pool:
24.1 MB/core @ ~360 GB/s = ~67 us -> DMA-bound.
"""

import sys

for _p in ("/opt/trn_rl_repo", "/root/.axon_site/_ro/trn_rl_repo"):
    if _p not in sys.path:
        sys.path.append(_p)

from contextlib import ExitStack

import numpy as np

import concourse.bass as bass
import concourse.tile as tile
from concourse import bacc, mybir
from concourse.bass_utils import run_bass_kernel_spmd
from concourse.masks import make_identity, make_upper_triangular

FP = mybir.dt.float32
F16 = mybir.dt.float16
OP = mybir.AluOpType
AF = mybir.ActivationFunctionType
N_CORES = 8
N_RAYS = 65536
S = 128
ONE_E_10 = 1.0e10
CLAMP = -6000.0  # pre-dn-scale clamp; -6000*dn stays fp16-finite, exp() ~ 0


def build_module(n_rays=N_RAYS // N_CORES, bufs=9, wbufs=16, mbufs=7, psbufs=4):
    J = n_rays // 128  # rays per partition (=64)
    G = 4  # ray-groups (j's) per block
    nblk = J // G  # 16
    W = G * S  # 512 free width

    nc = bacc.Bacc("TRN2", target_bir_lowering=False, debug=False)
    rf = nc.dram_tensor("radiance_field", [n_rays, S, 4], FP, kind="ExternalInput").ap()
    dv = nc.dram_tensor("depth_values", [n_rays, S], FP, kind="ExternalInput").ap()
    rd = nc.dram_tensor("ray_directions", [n_rays, 3], FP, kind="ExternalInput").ap()
    out = nc.dram_tensor("weights", [n_rays, S], F16, kind="ExternalOutput").ap()

    rf_v = rf.rearrange("(p j) s c -> p j (s c)", j=J)  # [128, J, 512]
    dv_v = dv.rearrange("(p j) s -> p j s", j=J)  # [128, J, S]
    out_v = out.rearrange("(p j) s -> p j s", j=J)
    rd_v = rd.rearrange("(p j) c -> p (j c)", j=J)  # [128, 3J]

    with tile.TileContext(nc) as tc, ExitStack() as ctx:
        consts = ctx.enter_context(tc.tile_pool(name="consts", bufs=1))
        rfp = ctx.enter_context(tc.tile_pool(name="rf", bufs=bufs))
        dvp = ctx.enter_context(tc.tile_pool(name="dv", bufs=bufs))
        ndp = ctx.enter_context(tc.tile_pool(name="nd", bufs=mbufs))
        mrp = ctx.enter_context(tc.tile_pool(name="mr", bufs=mbufs))
        m16p = ctx.enter_context(tc.tile_pool(name="m16", bufs=mbufs))
        mtp = ctx.enter_context(tc.tile_pool(name="mt", bufs=mbufs))
        tsp = ctx.enter_context(tc.tile_pool(name="ts", bufs=16))
        wpp = ctx.enter_context(tc.tile_pool(name="w", bufs=wbufs))
        psA = ctx.enter_context(tc.tile_pool(name="psA", bufs=psbufs, space="PSUM"))
        psB = ctx.enter_context(tc.tile_pool(name="psB", bufs=psbufs, space="PSUM"))

        # --- constants ---
        i16 = consts.tile([128, 128], F16, tag="i16")
        make_identity(nc, i16[:])
        u16 = consts.tile([128, 128], F16, tag="u16")
        make_upper_triangular(nc, u16[:], 1.0, diag=True)

        # --- prologue: dnpos[p, j] = |ray_dir(64p+j)| ---
        rdt = consts.tile([128, 3 * J], FP, tag="rdt")
        nc.gpsimd.dma_start(rdt[:], rd_v)
        sq = consts.tile([128, 3 * J], FP, tag="sq")
        nc.vector.tensor_mul(sq[:], rdt[:], rdt[:])
        sq3 = sq[:].rearrange("p (j c) -> p j c", c=3)
        dn2 = consts.tile([128, J], FP, tag="dn2")
        nc.vector.tensor_add(dn2[:], sq3[:, :, 0], sq3[:, :, 1])
        nc.vector.tensor_add(dn2[:], dn2[:], sq3[:, :, 2])
        dnpos = consts.tile([128, J], FP, tag="dnpos")
        nc.scalar.activation(dnpos[:], dn2[:], AF.Sqrt)

        # per-block live state, keyed by block index
        st = {}

        def do_load(b):
            j0 = b * G
            rf_t = rfp.tile([128, W * 4], FP, tag="rf")
            nc.sync.dma_start(
                rf_t[:].rearrange("p (j x) -> p j x", j=G), rf_v[:, j0 : j0 + G, :]
            )
            dv_t = dvp.tile([128, W], FP, tag="dv")
            nc.scalar.dma_start(
                dv_t[:].rearrange("p (j s) -> p j s", j=G), dv_v[:, j0 : j0 + G, :]
            )
            st[b] = {"rf": rf_t, "dv": dv_t}

        def do_m(b):
            s = st[b]
            dv_t = s.pop("dv")
            # nd[c] = dv[c] - dv[c+1] (negative dist); boundary cols -> -1e10
            nd = ndp.tile([128, W], FP, tag="nd")
            nc.gpsimd.tensor_sub(nd[:, 0 : W - 1], dv_t[:, 0 : W - 1], dv_t[:, 1:W])
            nc.gpsimd.memset(
                nd[:].rearrange("p (g s) -> p g s", s=S)[:, :, S - 1], -ONE_E_10
            )
            # mr = relu(sigma) * nd  (sigma strided x4 from packed rf)
            rf_t = s.pop("rf")
            sig = rf_t[:].rearrange("p (x c) -> p x c", c=4)[:, :, 3]
            mr = mrp.tile([128, W], FP, tag="mr")
            nc.vector.scalar_tensor_tensor(mr[:], sig, 0.0, nd[:], OP.max, OP.mult)
            # m16 = max(mr, -6000) * dn  (dn broadcast along s via stride-0 AP)
            m16 = m16p.tile([128, W], F16, tag="m16")
            dn_b = (
                dnpos[:, b * G : (b + 1) * G]
                .rearrange("p (g o) -> p g o", o=1)
                .broadcast_to([128, G, S])
            )  # stride-0 along s
            mr3 = mr[:].rearrange("p (g s) -> p g s", s=S)
            m163 = m16[:].rearrange("p (g s) -> p g s", s=S)
            nc.vector.scalar_tensor_tensor(m163, mr3, CLAMP, dn_b, OP.max, OP.mult)
            s["m16"] = m16

        def do_transpose(b):
            s = st[b]
            m16 = s.pop("m16")
            mt_ps = psA.tile([128, W], F16, tag="mt_ps")
            for g in range(G):
                sl = slice(S * g, S * (g + 1))
                nc.tensor.transpose(mt_ps[:, sl], m16[:, sl], i16[:])
            s["mt_ps"] = mt_ps

        def do_copy(b):
            s = st[b]
            mt_ps = s.pop("mt_ps")
            mt_sb = mtp.tile([128, W], F16, tag="mt_sb")
            nc.scalar.copy(mt_sb[:], mt_ps[:])
            s["mt_sb"] = mt_sb

        def do_mm(b):
            s = st[b]
            mt_sb = s.pop("mt_sb")
            min_ps = psB.tile([128, W], FP, tag="min_ps")
            for g in range(G):
                sl = slice(S * g, S * (g + 1))
                # out[r, s_out] = sum_s mt[s, r] * U[s, s_out]  (ray-major!)
                nc.tensor.matmul(min_ps[:, sl], mt_sb[:, sl], u16[:])
            s["min_ps"] = min_ps

        def do_exp(b):
            s = st[b]
            min_ps = s.pop("min_ps")
            t32 = tsp.tile([128, W], FP, tag="t32")
            nc.scalar.activation(t32[:], min_ps[:], AF.Exp)
            s["t32"] = t32

        def do_sub(b):
            s = st[b]
            t32 = s.pop("t32")
            w_t = wpp.tile([128, W], F16, tag="w")
            nc.gpsimd.tensor_sub(w_t[:, 1:W], t32[:, 0 : W - 1], t32[:, 1:W])
            w3 = w_t[:].rearrange("p (g s) -> p g s", s=S)[:, :, 0]
            t3 = t32[:].rearrange("p (g s) -> p g s", s=S)[:, :, 0]
            nc.gpsimd.tensor_scalar(w3, t3, -1.0, 1.0, OP.mult, OP.add)
            s["w"] = w_t

        def do_store(b):
            s = st.pop(b)
            w_t = s.pop("w")
            j0 = b * G
            nc.sync.dma_start(
                out_v[:, j0 : j0 + G, :], w_t[:].rearrange("p (g s) -> p g s", s=S)
            )

        # software-pipelined emission: stage lags keep each engine's
        # in-order stream free of long cross-engine waits
        stages = [
            (0, do_load),
            (1, do_m),
            (2, do_transpose),
            (2, do_copy),
            (3, do_mm),
            (3, do_exp),
        ]
        maxlag = max(lag for lag, _ in stages)
        for i in range(nblk + maxlag):
            for lag, fn in stages:
                b = i - lag
                if 0 <= b < nblk:
                    fn(b)
        # epilogue stages: lowest priority, fill engine idle slots without
        # ever blocking the load-paced main pipeline
        for b in range(nblk):
            do_sub(b)
            do_store(b)

    nc.compile()
    return nc


_NC_CACHE = {}


def get_module(n_rays=N_RAYS // N_CORES, **kw):
    key = (n_rays, tuple(sorted(kw.items())))
    if key not in _NC_CACHE:
        _NC_CACHE[key] = build_module(n_rays, **kw)
    return _NC_CACHE[key]


def run_spmd(radiance_field, depth_values, ray_directions, trace=False, **kw):
    nc = get_module(**kw)
    per = radiance_field.shape[0] // N_CORES
    in_maps = []
    for i in range(N_CORES):
        s = slice(i * per, (i + 1) * per)
        in_maps.append(
            {
                "radiance_field": np.ascontiguousarray(radiance_field[s]),
                "depth_values": np.ascontiguousarray(depth_values[s]),
                "ray_directions": np.ascontiguousarray(ray_directions[s]),
            }
        )
    res = run_bass_kernel_spmd(nc, in_maps, list(range(N_CORES)), trace=trace)
    out = np.concatenate([r["weights"].astype(np.float32) for r in res.results], axis=0)
    return out, res


def kernel(radiance_field, depth_values, ray_directions):
    out, _ = run_spmd(
        np.asarray(radiance_field, dtype=np.float32),
        np.asarray(depth_values, dtype=np.float32),
        np.asarray(ray_directions, dtype=np.float32),
    )
    return out
